# revision 1
# baseline (speedup 1.0000x reference)
"""Trainium2 Bass kernel v2 for nn_AdvancedGCN (3-layer GCN + BN + MHA(seq=1) + LN).

Key differences vs v1:
  - All static per-edge structure precomputed on host:
      * edges (+ self loops) sorted by (dest tile, src chunk); sel one-hot
        matrices with the full symmetric-norm coefficient c_e =
        dinv[src]*dinv[dst] folded in are streamed from DRAM (no on-device
        sel builds, no dinv anywhere on device).
      * layer-0 messages are host-gathered (x is static) and streamed
        sequentially -> no dma_gather for layer 0.
  - Transposed compute layout [feat -> partitions, nodes -> free]:
      * BN scale/shift/relu are single per-partition Scalar-engine ops
      * W applied as stationary lhsT; BN stats via free-dim reduces and a
        [128,2] AllReduce
      * residuals and h live in SBUF (no DRAM round trips)
  - Layers 1/2 tables rebuilt via 4 chunked AllGathers (pipelined with
    gathers); dma_gather calls are per (tile-batch, chunk) with large M,
    round-robin over SWDGE queues when enabled.
"""

import math
from dataclasses import dataclass, field

import numpy as np
import ml_dtypes

import concourse.bass as bass
import concourse.bacc as bacc
import concourse.tile as tile
from concourse import mybir
from concourse.bass_utils import run_bass_kernel_spmd

BF16 = ml_dtypes.bfloat16
P = 128
EPS_BN = 1e-5
EPS_LN = 1e-5


@dataclass
class Cfg:
    N: int = 100000
    E: int = 1600000
    D: int = 128
    C: int = 8            # cores
    B: int = 5            # dest tiles per batch (PSUM tiles)
    NQ: int = 1           # SWDGE queues to round-robin gathers over
    MAXI: int = 1024      # max idxs per dma_gather call
    sel_host: bool = True  # stream sel from host (else build on device)

    @property
    def NLOC(self):
        return self.N // self.C

    @property
    def T(self):
        return math.ceil(self.NLOC / P)

    @property
    def QSIZES(self):
        # AllGather piece sizes (local rows), multiples of P except the last
        t = self.T
        a = (t // 4) * P
        return [a, a, a, self.NLOC - 3 * a]

    @property
    def CHS(self):
        # gather-chunk sizes (= C * piece size)
        return [q * self.C for q in self.QSIZES]


@dataclass
class Plan:
    batches: list = field(default_factory=list)
    calls: list = field(default_factory=list)        # [b][k] -> dict
    tile_slots: list = field(default_factory=list)   # [b][k][t_local] -> nslots
    icols: int = 0
    scols: int = 0
    G: np.ndarray = None


def preprocess(cfg: Cfg, x, edge_index):
    """Host-side index/layout prep. Returns (plan, per_core list of dicts)."""
    N, C, T, B, NLOC = cfg.N, cfg.C, cfg.T, cfg.B, cfg.NLOC
    K = 4
    QS = cfg.QSIZES
    qs0 = QS[0]
    row = np.asarray(edge_index[0], dtype=np.int64)
    col = np.asarray(edge_index[1], dtype=np.int64)

    deg = np.bincount(col, minlength=N).astype(np.float64) + 1.0
    dinv = (1.0 / np.sqrt(deg)).astype(np.float64)

    ar = np.arange(N, dtype=np.int64)
    row_a = np.concatenate([row, ar])
    col_a = np.concatenate([col, ar])
    ce_a = (dinv[row_a] * dinv[col_a]).astype(np.float32)

    core = col_a // NLOC
    lcol = col_a - core * NLOC
    tl = lcol // P
    dwt = lcol - tl * P
    # chunk/piece of the SOURCE node, and its position in the gather table
    lrow = row_a % NLOC
    bounds = np.cumsum(QS)[:-1].astype(np.int64)
    rq = np.searchsorted(bounds, lrow, side="right")  # piece id 0..3
    qstart = np.concatenate([[0], bounds]).astype(np.int64)
    # position within chunk rq's gather table (size CHS[rq])
    pos = (row_a // NLOC) * np.array(QS, np.int64)[rq] + (lrow - qstart[rq])

    cid = ((core * T) + tl) * K + rq
    order = np.argsort(cid, kind="stable")
    s_pos = pos[order]
    s_dwt = dwt[order]
    s_ce = ce_a[order]
    s_row = row_a[order]
    cnt = np.bincount(cid, minlength=C * T * K).reshape(C, T, K)
    starts = np.zeros(C * T * K + 1, dtype=np.int64)
    np.cumsum(cnt.reshape(-1), out=starts[1:])

    G = np.ceil(cnt.max(axis=0) / P).astype(np.int64)  # [T, K] slots

    batches = [list(range(i, min(i + B, T))) for i in range(0, T, B)]
    plan = Plan(batches=batches, G=G)
    idx_off = 0
    slot_off = 0
    for b in batches:
        bc, bs = [], []
        for k in range(K):
            ns = int(sum(G[t, k] for t in b))
            M = P * ns
            bc.append({"M": M, "idx_off": idx_off, "slot_off": slot_off})
            bs.append([int(G[t, k]) for t in b])
            idx_off += M // 16
            slot_off += ns
        plan.calls.append(bc)
        plan.tile_slots.append(bs)
    plan.icols = idx_off
    plan.scols = slot_off
    S = plan.scols

    x32 = np.asarray(x, np.float32)

    per_core = []
    for c in range(C):
        # per (tile, chunk) padded edge arrays in slot order
        pos_pad = np.zeros(S * P, np.int64)
        dwt_pad = -np.ones(S * P, np.int64)
        ce_pad = np.zeros(S * P, np.float32)
        row_pad = np.zeros(S * P, np.int64)
        so = 0
        for bi, b in enumerate(batches):
            for k in range(K):
                for tj, t in enumerate(b):
                    g = int(G[t, k])
                    if g == 0:
                        continue
                    s0 = starts[(c * T + t) * K + k]
                    n = cnt[c, t, k]
                    sl = slice(so * P, so * P + n)
                    pos_pad[sl] = s_pos[s0:s0 + n]
                    dwt_pad[sl] = s_dwt[s0:s0 + n]
                    ce_pad[sl] = s_ce[s0:s0 + n]
                    row_pad[sl] = s_row[s0:s0 + n]
                    so += g
        assert so == S

        # IDX (int16 wrapped) for gather layers
        wi = pos_pad.reshape(S * P // 16, 16).T.astype(np.int16)
        IDX = np.tile(wi, (8, 1))
        assert IDX.shape == (P, plan.icols)

        # layer-0 messages: x rows in slot order -> [P, S*P] bf16
        m0 = x32[row_pad].astype(BF16)          # [S*P, D]
        m0[ce_pad == 0.0] = 0
        MSG0 = np.ascontiguousarray(
            m0.reshape(S, P, cfg.D).transpose(1, 0, 2).reshape(P, S * cfg.D))

        # sel stream: [P, S*P] bf16, sel[e, s*P + d] = ce if dwt==d
        SEL = np.zeros((S, P, P), np.float32)
        sidx = np.arange(S * P) // P
        eidx = np.arange(S * P) % P
        valid = dwt_pad >= 0
        SEL[sidx[valid], eidx[valid], dwt_pad[valid]] = ce_pad[valid]
        SEL = np.ascontiguousarray(
            SEL.transpose(1, 0, 2).reshape(P, S * P).astype(BF16))

        # transposed local x (residual for layer 0): [P(feat), T*P] bf16
        xT = np.zeros((P, T * P), np.float32)
        xT[:, :NLOC] = x32[c * NLOC:(c + 1) * NLOC].T
        per_core.append({"idx": IDX, "msg0": MSG0, "selstr": SEL,
                         "xT0": xT.astype(BF16)})
    return plan, per_core


def make_weight_inputs(cfg, W0, b0, W1, b1, W2, b2, g0, be0, g1, be1,
                      Wv, bv, Wo, bo, ln_g, ln_b):
    D = cfg.D
    f = lambda a: np.ascontiguousarray(np.asarray(a, np.float32))
    M2p = np.eye(D, dtype=np.float32) + f(Wv) @ f(Wo)
    bvo = f(bv) @ f(Wo) + f(bo)
    col = lambda a: f(a).reshape(D, 1)
    out = {
        "W0": f(W0), "W1": f(W1), "W2": f(W2),
        "m2p": M2p.astype(BF16),
        "b0c": col(b0), "b1c": col(b1), "b2c": col(b2),
        "g0c": col(g0), "be0c": col(be0), "g1c": col(g1), "be1c": col(be1),
        "bvob": np.tile(f(bvo).reshape(1, D), (P, 1)),
        "lngb": np.tile(f(ln_g).reshape(1, D), (P, 1)),
        "lnbb": np.tile(f(ln_b).reshape(1, D), (P, 1)),
        "identf": np.eye(P, dtype=np.float32),
        "identb": np.eye(P, dtype=np.float32).astype(BF16),
    }
    return out


def build_program(cfg: Cfg, plan: Plan):
    dt = mybir.dt
    f32 = dt.float32
    bf16 = dt.bfloat16
    N, D, T, NLOC, B = cfg.N, cfg.D, cfg.T, cfg.NLOC, cfg.B
    K = 4
    QS, CHS = cfg.QSIZES, cfg.CHS
    AF = mybir.ActivationFunctionType
    OP = mybir.AluOpType
    RG = [list(range(cfg.C))]
    S = plan.scols

    nc = bacc.Bacc("TRN2", target_bir_lowering=False, debug=False,
                   num_devices=cfg.C,
                   num_swdge_queues=max(cfg.NQ, 1))

    def inp(name, shape, dtype):
        return nc.dram_tensor(name, list(shape), dtype, kind="ExternalInput")

    IDXd = inp("idx", [P, max(plan.icols, 16)], dt.int16)
    MSG0d = inp("msg0", [P, S * P], bf16)
    SELd = inp("selstr", [P, S * P], bf16)
    xT0d = inp("xT0", [P, T * P], bf16)
    Wd = [inp(f"W{l}", [D, D], f32) for l in range(3)]
    m2pd = inp("m2p", [D, D], bf16)
    cols_d = {n: inp(n, [D, 1], f32)
              for n in ["b0c", "b1c", "b2c", "g0c", "be0c", "g1c", "be1c"]}
    bcasts_d = {n: inp(n, [P, D], f32) for n in ["bvob", "lngb", "lnbb"]}
    identfd = inp("identf", [P, P], f32)
    identbd = inp("identb", [P, P], bf16)

    # y stored transposed [feat, node]; host transposes back (free)
    y = nc.dram_tensor("y", [D, T * P], f32, kind="ExternalOutput")

    # per-piece table buffers for the 2 AllGathered layers
    aginp = [[nc.dram_tensor(f"agin{l}_{q}", [QS[q], D], bf16)
              if QS[q] > 0 else None for q in range(4)] for l in range(2)]
    xtabp = [[nc.dram_tensor(f"xtab{l}_{q}", [CHS[q], D], bf16,
                             addr_space="Shared")
              if CHS[q] > 0 else None for q in range(4)] for l in range(2)]
    stin = [nc.dram_tensor(f"stin{l}", [P, 2], f32) for l in range(2)]
    stout = [nc.dram_tensor(f"stout{l}", [P, 2], f32, addr_space="Shared")
             for l in range(2)]

    # piece boundaries in units of tiles (tile t -> piece p if fully inside)
    tpp = [QS[0] // P, QS[0] // P, QS[0] // P, T - 3 * (QS[0] // P)]
    tile_piece = []
    for pch, ntl in enumerate(tpp):
        tile_piece += [pch] * ntl

    maxns = max(c["M"] // P for bc in plan.calls for c in bc)

    from contextlib import ExitStack
    with tile.TileContext(nc) as tc, ExitStack() as ctx:
        cp = ctx.enter_context(tc.tile_pool(name="const", bufs=1))
        msgp = ctx.enter_context(tc.tile_pool(name="msg", bufs=2))
        selp = ctx.enter_context(tc.tile_pool(name="sel", bufs=2))
        agp = ctx.enter_context(tc.tile_pool(name="ag", bufs=3))
        ap_ = ctx.enter_context(tc.tile_pool(name="apply", bufs=3))
        smallp = ctx.enter_context(tc.tile_pool(name="small", bufs=2))
        psA = ctx.enter_context(tc.tile_pool(name="psA", bufs=B, space="PSUM"))
        psB = ctx.enter_context(tc.tile_pool(name="psB", bufs=1, space="PSUM"))

        def load(d, shape, dtype, tag):
            t = cp.tile(shape, dtype, tag=tag)
            nc.sync.dma_start(out=t[:], in_=d[:, :])
            return t

        IDX = load(IDXd, [P, max(plan.icols, 16)], dt.int16, "IDX")
        W_sb = [load(Wd[l], [D, D], f32, f"W{l}") for l in range(3)]
        m2p_sb = load(m2pd, [D, D], bf16, "m2p")
        cols = {n: load(d, [D, 1], f32, n) for n, d in cols_d.items()}
        bcasts = {n: load(d, [P, D], f32, n) for n, d in bcasts_d.items()}
        identf = load(identfd, [P, P], f32, "identf")
        identb = load(identbd, [P, P], bf16, "identb")

        # resident state
        hpreT = cp.tile([P, T * P], f32, tag="hpreT")
        nc.vector.memset(hpreT[:], 0.0)
        resT = cp.tile([P, T * P], bf16, tag="resT")      # residual in
        nc.sync.dma_start(out=resT[:], in_=xT0d[:, :])
        houtT = cp.tile([P, T * P], bf16, tag="houtT")    # h out (next res)
        eps1 = cp.tile([P, 1], f32, tag="eps1")
        nc.vector.memset(eps1[:], EPS_BN)
        epsl = cp.tile([P, 1], f32, tag="epsl")
        nc.vector.memset(epsl[:], EPS_LN)

        def nrows(t):
            return min(P, NLOC - t * P)

        def agg_phase(l):
            """messages x sel -> PSUM aggT per tile -> @W -> hpreT."""
            for bi, b in enumerate(plan.batches):
                pss = {t: psA.tile([P, P], f32, tag="agg", name=f"agg{l}_{t}")
                       for t in b}
                first = {t: True for t in b}
                last_k = {}
                for k in range(K):
                    for tj, t in enumerate(b):
                        if plan.tile_slots[bi][k][tj] > 0:
                            last_k[t] = k
                for k in range(K):
                    call = plan.calls[bi][k]
                    M, ns = call["M"], call["M"] // P
                    if M == 0:
                        continue
                    so = call["slot_off"]
                    msg = msgp.tile([P, ns, P], bf16, tag="msg",
                                    padded_shape=[P, maxns, P])
                    if l == 0:
                        nc.sync.dma_start(
                            out=msg[:, :, :],
                            in_=MSG0d[:, so * P:(so + ns) * P])
                    else:
                        MAXI = cfg.MAXI
                        io = call["idx_off"]
                        for j in range(0, M, MAXI):
                            Ms = min(MAXI, M - j)
                            nc.gpsimd.dma_gather(
                                out_ap=msg[:, j // P:(j + Ms) // P, :],
                                in_ap=xtabp[l - 1][k][0:CHS[k], :],
                                idxs_ap=IDX[:, io + j // 16:io + (j + Ms) // 16],
                                num_idxs=Ms, num_idxs_reg=Ms, elem_size=D,
                                queue_num=(bi * K + k) % cfg.NQ if cfg.NQ > 1 else 0)
                    sel = selp.tile([P, ns, P], bf16, tag="sel",
                                    padded_shape=[P, maxns, P])
                    nc.scalar.dma_start(
                        out=sel[:, :, :], in_=SELd[:, so * P:(so + ns) * P])
                    si = 0
                    for tj, t in enumerate(b):
                        g = plan.tile_slots[bi][k][tj]
                        for j in range(g):
                            nc.tensor.matmul(
                                pss[t][:], lhsT=msg[:, si, :],
                                rhs=sel[:, si, :],
                                start=first[t],
                                stop=(k == last_k[t] and j == g - 1))
                            first[t] = False
                            si += 1
                for t in b:
                    aggsb = agp.tile([P, P], f32, tag="aggsb")
                    nc.scalar.activation(aggsb[:], pss[t][:], AF.Copy)
                    ph = psB.tile([P, P], f32, tag="ph")
                    nc.tensor.matmul(ph[:], lhsT=W_sb[l][:], rhs=aggsb[:],
                                     start=True, stop=True)
                    if l < 2:
                        # conv bias b{l} is folded into the BN shift
                        nv = nrows(t)
                        nc.scalar.activation(hpreT[:, t * P:t * P + nv],
                                             ph[:, :nv], AF.Copy)
                    else:
                        out_phase_tile(t, ph)

        def out_phase_tile(t, ph):
            """ph [f, d] (+b2 per partition) -> y columns (transposed)."""
            phb = agp.tile([P, P], f32, tag="phb")
            nc.vector.tensor_scalar(phb[:], ph[:], cols["b2c"][:], None,
                                    op0=OP.add)
            nc.sync.dma_start(out=y[:, t * P:(t + 1) * P], in_=phb[:])

        def bn_phase(l):
            """stats over hpreT -> AllReduce -> Bsc/Bsh [P,1]."""
            CH = 2048
            s1 = smallp.tile([P, 1], f32, tag="s1")
            s2 = smallp.tile([P, 1], f32, tag="s2")
            nc.vector.memset(s1[:], 0.0)
            nc.vector.memset(s2[:], 0.0)
            nch = (NLOC + CH - 1) // CH
            for i in range(nch):
                c0 = i * CH
                cw = min(CH, NLOC - c0)
                p1 = smallp.tile([P, 1], f32, tag="p1")
                nc.vector.tensor_reduce(p1[:], hpreT[:, c0:c0 + cw],
                                        axis=mybir.AxisListType.X, op=OP.add)
                sq = ap_.tile([P, CH], f32, tag="sqch")
                p2 = smallp.tile([P, 1], f32, tag="p2")
                nc.scalar.activation(sq[:, :cw], hpreT[:, c0:c0 + cw],
                                     AF.Square, accum_out=p2[:])
                nc.vector.tensor_tensor(s1[:], s1[:], p1[:], op=OP.add)
                nc.vector.tensor_tensor(s2[:], s2[:], p2[:], op=OP.add)
            st = smallp.tile([P, 2], f32, tag="stcat")
            nc.vector.tensor_copy(st[:, 0:1], s1[:])
            nc.vector.tensor_copy(st[:, 1:2], s2[:])
            nc.sync.dma_start(out=stin[l][:, :], in_=st[:])
            nc.gpsimd.collective_compute(
                "AllReduce", OP.add, replica_groups=RG,
                ins=[stin[l].ap().opt()], outs=[stout[l].ap().opt()])
            stg = smallp.tile([P, 2], f32, tag="stg")
            nc.sync.dma_start(out=stg[:], in_=stout[l][:, :])
            invn = 1.0 / float(N)
            mu = smallp.tile([P, 1], f32, tag="mu")
            nc.scalar.activation(mu[:], stg[:, 0:1], AF.Copy, scale=invn)
            ex2 = smallp.tile([P, 1], f32, tag="ex2")
            nc.scalar.activation(ex2[:], stg[:, 1:2], AF.Copy, scale=invn)
            musq = smallp.tile([P, 1], f32, tag="musq")
            nc.scalar.activation(musq[:], mu[:], AF.Square)
            var = smallp.tile([P, 1], f32, tag="var")
            nc.vector.tensor_tensor(var[:], ex2[:], musq[:], op=OP.subtract)
            std = smallp.tile([P, 1], f32, tag="std")
            nc.scalar.activation(std[:], var[:], AF.Sqrt, bias=eps1[:])
            rstd = smallp.tile([P, 1], f32, tag="rstd")
            nc.vector.reciprocal(rstd[:], std[:])
            Bsc = smallp.tile([P, 1], f32, tag="Bsc")
            nc.vector.tensor_tensor(Bsc[:], cols[f"g{l}c"][:], rstd[:],
                                    op=OP.mult)
            # conv bias b{l} needs no handling: a per-feature constant shift
            # before BatchNorm cancels exactly (BN removes the mean).
            ms = smallp.tile([P, 1], f32, tag="ms")
            nc.vector.tensor_tensor(ms[:], mu[:], Bsc[:], op=OP.mult)
            Bsh = smallp.tile([P, 1], f32, tag="Bsh")
            nc.vector.tensor_tensor(Bsh[:], cols[f"be{l}c"][:], ms[:],
                                    op=OP.subtract)
            return Bsc, Bsh

        def apply_phase(l, Bsc, Bsh):
            """houtT = relu(Bsc*hpreT+Bsh) + resT; l==0: also write table."""
            for t in range(T):
                r = ap_.tile([P, P], f32, tag="r")
                nc.scalar.activation(r[:], hpreT[:, t * P:(t + 1) * P],
                                     AF.Relu, scale=Bsc[:], bias=Bsh[:])
                nc.vector.tensor_tensor(houtT[:, t * P:(t + 1) * P], r[:],
                                        resT[:, t * P:(t + 1) * P], op=OP.add)
                if l == 0:
                    write_table_tile(0, t)

        def write_table_tile(l, t):
            """houtT tile [f, n] -> transpose -> agin piece rows (bf16)."""
            pt = psB.tile([P, P], bf16, tag="ptb")
            nc.tensor.transpose(pt[:], houtT[:, t * P:(t + 1) * P], identb[:])
            xt = ap_.tile([P, P], bf16, tag="xt")
            nc.vector.tensor_copy(xt[:], pt[:])
            pch = tile_piece[t]
            r0 = t * P - sum(QS[:pch])
            nv = nrows(t)
            nc.sync.dma_start(out=aginp[l][pch][r0:r0 + nv, :], in_=xt[:nv, :])

        def allgather_tables(l):
            for q in range(4):
                if QS[q] == 0:
                    continue
                nc.gpsimd.collective_compute(
                    "AllGather", OP.bypass, replica_groups=RG,
                    ins=[aginp[l][q].ap().opt()],
                    outs=[xtabp[l][q].ap().opt()])

        def mha_phase():
            """xt3 = LN(h2 @ M2p + bvo) -> agin[1] (node-major directly)."""
            for t in range(T):
                sp = psB.tile([P, P], f32, tag="sps")
                nc.tensor.matmul(sp[:], lhsT=houtT[:, t * P:(t + 1) * P],
                                 rhs=m2p_sb[:], start=True, stop=True)
                s = ap_.tile([P, P], f32, tag="s")
                nc.vector.tensor_tensor(s[:], sp[:], bcasts["bvob"][:],
                                        op=OP.add)
                msum = ap_.tile([P, 1], f32, tag="msum")
                nc.vector.tensor_reduce(msum[:], s[:], axis=mybir.AxisListType.X,
                                        op=OP.add)
                mu = ap_.tile([P, 1], f32, tag="lmu")
                nc.scalar.activation(mu[:], msum[:], AF.Copy, scale=1.0 / D)
                cen = ap_.tile([P, P], f32, tag="cen")
                nc.vector.tensor_scalar(cen[:], s[:], mu[:], None,
                                        op0=OP.subtract)
                vsum = ap_.tile([P, 1], f32, tag="vsum")
                csq = ap_.tile([P, P], f32, tag="csq")
                nc.scalar.activation(csq[:], cen[:], AF.Square,
                                     accum_out=vsum[:])
                std = ap_.tile([P, 1], f32, tag="lstd")
                nc.scalar.activation(std[:], vsum[:], AF.Sqrt, bias=epsl[:],
                                     scale=1.0 / D)
                rstd = ap_.tile([P, 1], f32, tag="lrstd")
                nc.vector.reciprocal(rstd[:], std[:])
                nrm = ap_.tile([P, P], f32, tag="nrm")
                nc.vector.tensor_scalar(nrm[:], cen[:], rstd[:], None,
                                        op0=OP.mult)
                nc.vector.tensor_tensor(nrm[:], nrm[:], bcasts["lngb"][:],
                                        op=OP.mult)
                xt = ap_.tile([P, P], bf16, tag="xt3")
                nc.vector.tensor_tensor(xt[:], nrm[:], bcasts["lnbb"][:],
                                        op=OP.add)
                pch = tile_piece[t]
                r0 = t * P - sum(QS[:pch])
                nv = nrows(t)
                nc.sync.dma_start(out=aginp[1][pch][r0:r0 + nv, :],
                                  in_=xt[:nv, :])

        # ---------------- layer 0 ----------------
        agg_phase(0)
        Bsc, Bsh = bn_phase(0)
        apply_phase(0, Bsc, Bsh)
        allgather_tables(0)
        nc.vector.tensor_copy(resT[:], houtT[:])
        # ---------------- layer 1 ----------------
        agg_phase(1)
        Bsc, Bsh = bn_phase(1)
        apply_phase(1, Bsc, Bsh)
        mha_phase()
        allgather_tables(1)
        # ---------------- layer 2 (output) ----------------
        agg_phase(2)

    nc.compile()
    return nc


_CACHE = {}


def _get_program(cfg, plan):
    key = (cfg.N, cfg.E, cfg.C, cfg.B, cfg.NQ, cfg.MAXI, plan.icols, plan.scols)
    if key not in _CACHE:
        _CACHE[key] = build_program(cfg, plan)
    return _CACHE[key]


def run(cfg, inputs, trace=False):
    x = np.asarray(inputs["x"], np.float32)
    edge_index = np.asarray(inputs["edge_index"])
    plan, per_core = preprocess(cfg, x, edge_index)
    wts = make_weight_inputs(
        cfg, inputs["W0"], inputs["b0"], inputs["W1"], inputs["b1"],
        inputs["W2"], inputs["b2"], inputs["g0"], inputs["be0"],
        inputs["g1"], inputs["be1"], inputs["Wv"], inputs["bv"],
        inputs["Wo"], inputs["bo"], inputs["ln_g"], inputs["ln_b"])

    nc = _get_program(cfg, plan)

    in_maps = []
    for c in range(cfg.C):
        m = dict(wts)
        m.update(per_core[c])
        if plan.icols == 0:
            m["idx"] = np.zeros((P, 16), np.int16)
        in_maps.append(m)

    res = run_bass_kernel_spmd(nc, in_maps, core_ids=list(range(cfg.C)),
                               trace=trace)
    yfull = np.concatenate([res.results[c]["y"].T[:cfg.NLOC]
                            for c in range(cfg.C)], axis=0)
    return yfull.astype(np.float32), res


def kernel(**inputs) -> np.ndarray:
    cfg = Cfg()
    yfull, _ = run(cfg, inputs)
    return yfull



# revision 3
# speedup vs baseline: 1.1411x; 1.1411x over previous
"""Trainium2 Bass kernel v2 for nn_AdvancedGCN (3-layer GCN + BN + MHA(seq=1) + LN).

Key differences vs v1:
  - All static per-edge structure precomputed on host:
      * edges (+ self loops) sorted by (dest tile, src chunk); sel one-hot
        matrices with the full symmetric-norm coefficient c_e =
        dinv[src]*dinv[dst] folded in are streamed from DRAM (no on-device
        sel builds, no dinv anywhere on device).
      * layer-0 messages are host-gathered (x is static) and streamed
        sequentially -> no dma_gather for layer 0.
  - Transposed compute layout [feat -> partitions, nodes -> free]:
      * BN scale/shift/relu are single per-partition Scalar-engine ops
      * W applied as stationary lhsT; BN stats via free-dim reduces and a
        [128,2] AllReduce
      * residuals and h live in SBUF (no DRAM round trips)
  - Layers 1/2 tables rebuilt via 4 chunked AllGathers (pipelined with
    gathers); dma_gather calls are per (tile-batch, chunk) with large M,
    round-robin over SWDGE queues when enabled.
"""

import math
from dataclasses import dataclass, field

import numpy as np
import ml_dtypes

import concourse.bass as bass
import concourse.bacc as bacc
import concourse.tile as tile
from concourse import mybir
from concourse.bass_utils import run_bass_kernel_spmd

BF16 = ml_dtypes.bfloat16
P = 128
EPS_BN = 1e-5
EPS_LN = 1e-5


@dataclass
class Cfg:
    N: int = 100000
    E: int = 1600000
    D: int = 128
    C: int = 8            # cores
    B: int = 5            # dest tiles per batch (PSUM tiles)
    NQ: int = 1           # SWDGE queues to round-robin gathers over
    MAXI: int = 1024      # max idxs per dma_gather call
    sel_host: bool = True  # stream sel from host (else build on device)

    @property
    def NLOC(self):
        return self.N // self.C

    @property
    def T(self):
        return math.ceil(self.NLOC / P)

    @property
    def QSIZES(self):
        # AllGather piece sizes (local rows), multiples of P except the last
        t = self.T
        a = (t // 4) * P
        return [a, a, a, self.NLOC - 3 * a]

    @property
    def CHS(self):
        # gather-chunk sizes (= C * piece size)
        return [q * self.C for q in self.QSIZES]


@dataclass
class Plan:
    batches: list = field(default_factory=list)
    calls: list = field(default_factory=list)        # [b][k] -> dict
    tile_slots: list = field(default_factory=list)   # [b][k][t_local] -> nslots
    icols: int = 0
    scols: int = 0
    G: np.ndarray = None


def preprocess(cfg: Cfg, x, edge_index):
    """Host-side index/layout prep. Returns (plan, per_core list of dicts)."""
    N, C, T, B, NLOC = cfg.N, cfg.C, cfg.T, cfg.B, cfg.NLOC
    K = 4
    QS = cfg.QSIZES
    qs0 = QS[0]
    row = np.asarray(edge_index[0], dtype=np.int64)
    col = np.asarray(edge_index[1], dtype=np.int64)

    deg = np.bincount(col, minlength=N).astype(np.float64) + 1.0
    dinv = (1.0 / np.sqrt(deg)).astype(np.float64)

    ar = np.arange(N, dtype=np.int64)
    row_a = np.concatenate([row, ar])
    col_a = np.concatenate([col, ar])
    ce_a = (dinv[row_a] * dinv[col_a]).astype(np.float32)

    core = col_a // NLOC
    lcol = col_a - core * NLOC
    tl = lcol // P
    dwt = lcol - tl * P
    # chunk/piece of the SOURCE node, and its position in the gather table
    lrow = row_a % NLOC
    bounds = np.cumsum(QS)[:-1].astype(np.int64)
    rq = np.searchsorted(bounds, lrow, side="right")  # piece id 0..3
    qstart = np.concatenate([[0], bounds]).astype(np.int64)
    # position within chunk rq's gather table (size CHS[rq])
    pos = (row_a // NLOC) * np.array(QS, np.int64)[rq] + (lrow - qstart[rq])

    cid = ((core * T) + tl) * K + rq
    order = np.argsort(cid, kind="stable")
    s_pos = pos[order]
    s_dwt = dwt[order]
    s_ce = ce_a[order]
    s_row = row_a[order]
    cnt = np.bincount(cid, minlength=C * T * K).reshape(C, T, K)
    starts = np.zeros(C * T * K + 1, dtype=np.int64)
    np.cumsum(cnt.reshape(-1), out=starts[1:])

    G = np.ceil(cnt.max(axis=0) / P).astype(np.int64)  # [T, K] slots

    batches = [list(range(i, min(i + B, T))) for i in range(0, T, B)]
    plan = Plan(batches=batches, G=G)
    idx_off = 0
    slot_off = 0
    for b in batches:
        bc, bs = [], []
        for k in range(K):
            ns = int(sum(G[t, k] for t in b))
            M = P * ns
            bc.append({"M": M, "idx_off": idx_off, "slot_off": slot_off})
            bs.append([int(G[t, k]) for t in b])
            idx_off += M // 16
            slot_off += ns
        plan.calls.append(bc)
        plan.tile_slots.append(bs)
    plan.icols = idx_off
    plan.scols = slot_off
    S = plan.scols

    x32 = np.asarray(x, np.float32)

    per_core = []
    for c in range(C):
        # per (tile, chunk) padded edge arrays in slot order
        pos_pad = np.zeros(S * P, np.int64)
        dwt_pad = -np.ones(S * P, np.int64)
        ce_pad = np.zeros(S * P, np.float32)
        row_pad = np.zeros(S * P, np.int64)
        so = 0
        for bi, b in enumerate(batches):
            for k in range(K):
                for tj, t in enumerate(b):
                    g = int(G[t, k])
                    if g == 0:
                        continue
                    s0 = starts[(c * T + t) * K + k]
                    n = cnt[c, t, k]
                    sl = slice(so * P, so * P + n)
                    pos_pad[sl] = s_pos[s0:s0 + n]
                    dwt_pad[sl] = s_dwt[s0:s0 + n]
                    ce_pad[sl] = s_ce[s0:s0 + n]
                    row_pad[sl] = s_row[s0:s0 + n]
                    so += g
        assert so == S

        # IDX (int16 wrapped) for gather layers
        wi = pos_pad.reshape(S * P // 16, 16).T.astype(np.int16)
        IDX = np.tile(wi, (8, 1))
        assert IDX.shape == (P, plan.icols)

        # layer-0 messages: x rows in slot order -> [P, S*P] bf16
        m0 = x32[row_pad].astype(BF16)          # [S*P, D]
        m0[ce_pad == 0.0] = 0
        MSG0 = np.ascontiguousarray(
            m0.reshape(S, P, cfg.D).transpose(1, 0, 2).reshape(P, S * cfg.D))

        # sel stream: [P, S*P] bf16, sel[e, s*P + d] = ce if dwt==d
        SEL = np.zeros((S, P, P), np.float32)
        sidx = np.arange(S * P) // P
        eidx = np.arange(S * P) % P
        valid = dwt_pad >= 0
        SEL[sidx[valid], eidx[valid], dwt_pad[valid]] = ce_pad[valid]
        SEL = np.ascontiguousarray(
            SEL.transpose(1, 0, 2).reshape(P, S * P).astype(BF16))

        # transposed local x (residual for layer 0): [P(feat), T*P] bf16
        xT = np.zeros((P, T * P), np.float32)
        xT[:, :NLOC] = x32[c * NLOC:(c + 1) * NLOC].T
        per_core.append({"idx": IDX, "msg0": MSG0, "selstr": SEL,
                         "xT0": xT.astype(BF16)})
    return plan, per_core


def make_weight_inputs(cfg, W0, b0, W1, b1, W2, b2, g0, be0, g1, be1,
                      Wv, bv, Wo, bo, ln_g, ln_b):
    D = cfg.D
    f = lambda a: np.ascontiguousarray(np.asarray(a, np.float32))
    M2p = np.eye(D, dtype=np.float32) + f(Wv) @ f(Wo)
    bvo = f(bv) @ f(Wo) + f(bo)
    col = lambda a: f(a).reshape(D, 1)
    out = {
        "W0": f(W0), "W1": f(W1), "W2": f(W2),
        "m2p": M2p.astype(BF16),
        "b0c": col(b0), "b1c": col(b1), "b2c": col(b2),
        "g0c": col(g0), "be0c": col(be0), "g1c": col(g1), "be1c": col(be1),
        "bvob": np.tile(f(bvo).reshape(1, D), (P, 1)),
        "lngb": np.tile(f(ln_g).reshape(1, D), (P, 1)),
        "lnbb": np.tile(f(ln_b).reshape(1, D), (P, 1)),
        "identf": np.eye(P, dtype=np.float32),
        "identb": np.eye(P, dtype=np.float32).astype(BF16),
    }
    return out


def build_program(cfg: Cfg, plan: Plan):
    dt = mybir.dt
    f32 = dt.float32
    bf16 = dt.bfloat16
    N, D, T, NLOC, B = cfg.N, cfg.D, cfg.T, cfg.NLOC, cfg.B
    K = 4
    QS, CHS = cfg.QSIZES, cfg.CHS
    AF = mybir.ActivationFunctionType
    OP = mybir.AluOpType
    RG = [list(range(cfg.C))]
    S = plan.scols

    nc = bacc.Bacc("TRN2", target_bir_lowering=False, debug=False,
                   num_devices=cfg.C,
                   num_swdge_queues=max(cfg.NQ, 1))

    def inp(name, shape, dtype):
        return nc.dram_tensor(name, list(shape), dtype, kind="ExternalInput")

    IDXd = inp("idx", [P, max(plan.icols, 16)], dt.int16)
    MSG0d = inp("msg0", [P, S * P], bf16)
    SELd = inp("selstr", [P, S * P], bf16)
    xT0d = inp("xT0", [P, T * P], bf16)
    Wd = [inp(f"W{l}", [D, D], f32) for l in range(3)]
    m2pd = inp("m2p", [D, D], bf16)
    cols_d = {n: inp(n, [D, 1], f32)
              for n in ["b0c", "b1c", "b2c", "g0c", "be0c", "g1c", "be1c"]}
    bcasts_d = {n: inp(n, [P, D], f32) for n in ["bvob", "lngb", "lnbb"]}
    identfd = inp("identf", [P, P], f32)
    identbd = inp("identb", [P, P], bf16)

    # y stored transposed [feat, node]; host transposes back (free)
    y = nc.dram_tensor("y", [D, T * P], f32, kind="ExternalOutput")

    # per-piece table buffers for the 2 AllGathered layers
    aginp = [[nc.dram_tensor(f"agin{l}_{q}", [QS[q], D], bf16)
              if QS[q] > 0 else None for q in range(4)] for l in range(2)]
    xtabp = [[nc.dram_tensor(f"xtab{l}_{q}", [CHS[q], D], bf16,
                             addr_space="Shared")
              if CHS[q] > 0 else None for q in range(4)] for l in range(2)]
    stin = [nc.dram_tensor(f"stin{l}", [P, 2], f32) for l in range(2)]
    stout = [nc.dram_tensor(f"stout{l}", [P, 2], f32, addr_space="Shared")
             for l in range(2)]

    # piece boundaries in units of tiles (tile t -> piece p if fully inside)
    tpp = [QS[0] // P, QS[0] // P, QS[0] // P, T - 3 * (QS[0] // P)]
    tile_piece = []
    for pch, ntl in enumerate(tpp):
        tile_piece += [pch] * ntl

    maxns = max(c["M"] // P for bc in plan.calls for c in bc)

    from contextlib import ExitStack
    with tile.TileContext(nc) as tc, ExitStack() as ctx:
        cp = ctx.enter_context(tc.tile_pool(name="const", bufs=1))
        msgp = ctx.enter_context(tc.tile_pool(name="msg", bufs=2))
        selp = ctx.enter_context(tc.tile_pool(name="sel", bufs=2))
        agp = ctx.enter_context(tc.tile_pool(name="ag", bufs=3))
        ap_ = ctx.enter_context(tc.tile_pool(name="apply", bufs=3))
        smallp = ctx.enter_context(tc.tile_pool(name="small", bufs=2))
        psA = ctx.enter_context(tc.tile_pool(name="psA", bufs=B, space="PSUM"))
        psB = ctx.enter_context(tc.tile_pool(name="psB", bufs=1, space="PSUM"))

        def load(d, shape, dtype, tag):
            t = cp.tile(shape, dtype, tag=tag)
            nc.sync.dma_start(out=t[:], in_=d[:, :])
            return t

        IDX = load(IDXd, [P, max(plan.icols, 16)], dt.int16, "IDX")
        W_sb = [load(Wd[l], [D, D], f32, f"W{l}") for l in range(3)]
        m2p_sb = load(m2pd, [D, D], bf16, "m2p")
        cols = {n: load(d, [D, 1], f32, n) for n, d in cols_d.items()}
        bcasts = {n: load(d, [P, D], f32, n) for n, d in bcasts_d.items()}
        identf = load(identfd, [P, P], f32, "identf")
        identb = load(identbd, [P, P], bf16, "identb")

        # resident state
        hpreT = cp.tile([P, T * P], f32, tag="hpreT")
        nc.vector.memset(hpreT[:], 0.0)
        resT = cp.tile([P, T * P], bf16, tag="resT")      # residual in
        nc.sync.dma_start(out=resT[:], in_=xT0d[:, :])
        houtT = cp.tile([P, T * P], bf16, tag="houtT")    # h out (next res)
        eps1 = cp.tile([P, 1], f32, tag="eps1")
        nc.vector.memset(eps1[:], EPS_BN)
        epsl = cp.tile([P, 1], f32, tag="epsl")
        nc.vector.memset(epsl[:], EPS_LN)

        def nrows(t):
            return min(P, NLOC - t * P)

        self_counter = [0]

        def agg_phase(l):
            """messages x sel -> PSUM aggT per tile -> @W -> hpreT."""
            for bi, b in enumerate(plan.batches):
                pss = {t: psA.tile([P, P], f32, tag="agg", name=f"agg{l}_{t}")
                       for t in b}
                first = {t: True for t in b}
                last_k = {}
                for k in range(K):
                    for tj, t in enumerate(b):
                        if plan.tile_slots[bi][k][tj] > 0:
                            last_k[t] = k
                for k in range(K):
                    call = plan.calls[bi][k]
                    M, ns = call["M"], call["M"] // P
                    if M == 0:
                        continue
                    so = call["slot_off"]
                    msg = msgp.tile([P, ns, P], bf16, tag="msg",
                                    padded_shape=[P, maxns, P])
                    if l == 0:
                        nc.sync.dma_start(
                            out=msg[:, :, :],
                            in_=MSG0d[:, so * P:(so + ns) * P])
                    else:
                        MAXI = cfg.MAXI
                        io = call["idx_off"]
                        for j in range(0, M, MAXI):
                            Ms = min(MAXI, M - j)
                            qn = self_counter[0] % cfg.NQ if cfg.NQ > 1 else 0
                            self_counter[0] += 1
                            nc.gpsimd.dma_gather(
                                out_ap=msg[:, j // P:(j + Ms) // P, :],
                                in_ap=xtabp[l - 1][k][0:CHS[k], :],
                                idxs_ap=IDX[:, io + j // 16:io + (j + Ms) // 16],
                                num_idxs=Ms, num_idxs_reg=Ms, elem_size=D,
                                queue_num=qn)
                    sel = selp.tile([P, ns, P], bf16, tag="sel",
                                    padded_shape=[P, maxns, P])
                    nc.scalar.dma_start(
                        out=sel[:, :, :], in_=SELd[:, so * P:(so + ns) * P])
                    si = 0
                    for tj, t in enumerate(b):
                        g = plan.tile_slots[bi][k][tj]
                        for j in range(g):
                            nc.tensor.matmul(
                                pss[t][:], lhsT=msg[:, si, :],
                                rhs=sel[:, si, :],
                                start=first[t],
                                stop=(k == last_k[t] and j == g - 1))
                            first[t] = False
                            si += 1
                for t in b:
                    aggsb = agp.tile([P, P], f32, tag="aggsb")
                    nc.scalar.activation(aggsb[:], pss[t][:], AF.Copy)
                    ph = psB.tile([P, P], f32, tag="ph")
                    nc.tensor.matmul(ph[:], lhsT=W_sb[l][:], rhs=aggsb[:],
                                     start=True, stop=True)
                    if l < 2:
                        # conv bias b{l} is folded into the BN shift
                        nv = nrows(t)
                        nc.scalar.activation(hpreT[:, t * P:t * P + nv],
                                             ph[:, :nv], AF.Copy)
                    else:
                        out_phase_tile(t, ph)

        def out_phase_tile(t, ph):
            """ph [f, d] (+b2 per partition) -> y columns (transposed)."""
            phb = agp.tile([P, P], f32, tag="phb")
            nc.vector.tensor_scalar(phb[:], ph[:], cols["b2c"][:], None,
                                    op0=OP.add)
            nc.sync.dma_start(out=y[:, t * P:(t + 1) * P], in_=phb[:])

        def bn_phase(l):
            """stats over hpreT -> AllReduce -> Bsc/Bsh [P,1]."""
            CH = 2048
            s1 = smallp.tile([P, 1], f32, tag="s1")
            s2 = smallp.tile([P, 1], f32, tag="s2")
            nc.vector.memset(s1[:], 0.0)
            nc.vector.memset(s2[:], 0.0)
            nch = (NLOC + CH - 1) // CH
            for i in range(nch):
                c0 = i * CH
                cw = min(CH, NLOC - c0)
                p1 = smallp.tile([P, 1], f32, tag="p1")
                nc.vector.tensor_reduce(p1[:], hpreT[:, c0:c0 + cw],
                                        axis=mybir.AxisListType.X, op=OP.add)
                sq = ap_.tile([P, CH], f32, tag="sqch")
                p2 = smallp.tile([P, 1], f32, tag="p2")
                nc.scalar.activation(sq[:, :cw], hpreT[:, c0:c0 + cw],
                                     AF.Square, accum_out=p2[:])
                nc.vector.tensor_tensor(s1[:], s1[:], p1[:], op=OP.add)
                nc.vector.tensor_tensor(s2[:], s2[:], p2[:], op=OP.add)
            st = smallp.tile([P, 2], f32, tag="stcat")
            nc.vector.tensor_copy(st[:, 0:1], s1[:])
            nc.vector.tensor_copy(st[:, 1:2], s2[:])
            nc.sync.dma_start(out=stin[l][:, :], in_=st[:])
            nc.gpsimd.collective_compute(
                "AllReduce", OP.add, replica_groups=RG,
                ins=[stin[l].ap().opt()], outs=[stout[l].ap().opt()])
            stg = smallp.tile([P, 2], f32, tag="stg")
            nc.sync.dma_start(out=stg[:], in_=stout[l][:, :])
            invn = 1.0 / float(N)
            mu = smallp.tile([P, 1], f32, tag="mu")
            nc.scalar.activation(mu[:], stg[:, 0:1], AF.Copy, scale=invn)
            ex2 = smallp.tile([P, 1], f32, tag="ex2")
            nc.scalar.activation(ex2[:], stg[:, 1:2], AF.Copy, scale=invn)
            musq = smallp.tile([P, 1], f32, tag="musq")
            nc.scalar.activation(musq[:], mu[:], AF.Square)
            var = smallp.tile([P, 1], f32, tag="var")
            nc.vector.tensor_tensor(var[:], ex2[:], musq[:], op=OP.subtract)
            std = smallp.tile([P, 1], f32, tag="std")
            nc.scalar.activation(std[:], var[:], AF.Sqrt, bias=eps1[:])
            rstd = smallp.tile([P, 1], f32, tag="rstd")
            nc.vector.reciprocal(rstd[:], std[:])
            Bsc = smallp.tile([P, 1], f32, tag="Bsc")
            nc.vector.tensor_tensor(Bsc[:], cols[f"g{l}c"][:], rstd[:],
                                    op=OP.mult)
            # conv bias b{l} needs no handling: a per-feature constant shift
            # before BatchNorm cancels exactly (BN removes the mean).
            ms = smallp.tile([P, 1], f32, tag="ms")
            nc.vector.tensor_tensor(ms[:], mu[:], Bsc[:], op=OP.mult)
            Bsh = smallp.tile([P, 1], f32, tag="Bsh")
            nc.vector.tensor_tensor(Bsh[:], cols[f"be{l}c"][:], ms[:],
                                    op=OP.subtract)
            return Bsc, Bsh

        def apply_phase(l, Bsc, Bsh):
            """houtT = relu(Bsc*hpreT+Bsh) + resT; l==0: also write table."""
            for t in range(T):
                r = ap_.tile([P, P], f32, tag="r")
                nc.scalar.activation(r[:], hpreT[:, t * P:(t + 1) * P],
                                     AF.Relu, scale=Bsc[:], bias=Bsh[:])
                nc.vector.tensor_tensor(houtT[:, t * P:(t + 1) * P], r[:],
                                        resT[:, t * P:(t + 1) * P], op=OP.add)
                if l == 0:
                    write_table_tile(0, t)

        def write_table_tile(l, t):
            """houtT tile [f, n] -> transpose -> agin piece rows (bf16)."""
            pt = psB.tile([P, P], bf16, tag="ptb")
            nc.tensor.transpose(pt[:], houtT[:, t * P:(t + 1) * P], identb[:])
            xt = ap_.tile([P, P], bf16, tag="xt")
            nc.vector.tensor_copy(xt[:], pt[:])
            pch = tile_piece[t]
            r0 = t * P - sum(QS[:pch])
            nv = nrows(t)
            nc.sync.dma_start(out=aginp[l][pch][r0:r0 + nv, :], in_=xt[:nv, :])

        def allgather_tables(l):
            for q in range(4):
                if QS[q] == 0:
                    continue
                nc.gpsimd.collective_compute(
                    "AllGather", OP.bypass, replica_groups=RG,
                    ins=[aginp[l][q].ap().opt()],
                    outs=[xtabp[l][q].ap().opt()])

        def mha_phase():
            """xt3 = LN(h2 @ M2p + bvo) -> agin[1] (node-major directly)."""
            for t in range(T):
                sp = psB.tile([P, P], f32, tag="sps")
                nc.tensor.matmul(sp[:], lhsT=houtT[:, t * P:(t + 1) * P],
                                 rhs=m2p_sb[:], start=True, stop=True)
                s = ap_.tile([P, P], f32, tag="s")
                nc.vector.tensor_tensor(s[:], sp[:], bcasts["bvob"][:],
                                        op=OP.add)
                msum = ap_.tile([P, 1], f32, tag="msum")
                nc.vector.tensor_reduce(msum[:], s[:], axis=mybir.AxisListType.X,
                                        op=OP.add)
                mu = ap_.tile([P, 1], f32, tag="lmu")
                nc.scalar.activation(mu[:], msum[:], AF.Copy, scale=1.0 / D)
                cen = ap_.tile([P, P], f32, tag="cen")
                nc.vector.tensor_scalar(cen[:], s[:], mu[:], None,
                                        op0=OP.subtract)
                vsum = ap_.tile([P, 1], f32, tag="vsum")
                csq = ap_.tile([P, P], f32, tag="csq")
                nc.scalar.activation(csq[:], cen[:], AF.Square,
                                     accum_out=vsum[:])
                std = ap_.tile([P, 1], f32, tag="lstd")
                nc.scalar.activation(std[:], vsum[:], AF.Sqrt, bias=epsl[:],
                                     scale=1.0 / D)
                rstd = ap_.tile([P, 1], f32, tag="lrstd")
                nc.vector.reciprocal(rstd[:], std[:])
                nrm = ap_.tile([P, P], f32, tag="nrm")
                nc.vector.tensor_scalar(nrm[:], cen[:], rstd[:], None,
                                        op0=OP.mult)
                nc.vector.tensor_tensor(nrm[:], nrm[:], bcasts["lngb"][:],
                                        op=OP.mult)
                xt = ap_.tile([P, P], bf16, tag="xt3")
                nc.vector.tensor_tensor(xt[:], nrm[:], bcasts["lnbb"][:],
                                        op=OP.add)
                pch = tile_piece[t]
                r0 = t * P - sum(QS[:pch])
                nv = nrows(t)
                nc.sync.dma_start(out=aginp[1][pch][r0:r0 + nv, :],
                                  in_=xt[:nv, :])

        # ---------------- layer 0 ----------------
        agg_phase(0)
        Bsc, Bsh = bn_phase(0)
        apply_phase(0, Bsc, Bsh)
        allgather_tables(0)
        nc.vector.tensor_copy(resT[:], houtT[:])
        # ---------------- layer 1 ----------------
        agg_phase(1)
        Bsc, Bsh = bn_phase(1)
        apply_phase(1, Bsc, Bsh)
        mha_phase()
        allgather_tables(1)
        # ---------------- layer 2 (output) ----------------
        agg_phase(2)

    nc.compile()
    return nc


_CACHE = {}


def _get_program(cfg, plan):
    key = (cfg.N, cfg.E, cfg.C, cfg.B, cfg.NQ, cfg.MAXI, plan.icols, plan.scols)
    if key not in _CACHE:
        _CACHE[key] = build_program(cfg, plan)
    return _CACHE[key]


def run(cfg, inputs, trace=False):
    x = np.asarray(inputs["x"], np.float32)
    edge_index = np.asarray(inputs["edge_index"])
    plan, per_core = preprocess(cfg, x, edge_index)
    wts = make_weight_inputs(
        cfg, inputs["W0"], inputs["b0"], inputs["W1"], inputs["b1"],
        inputs["W2"], inputs["b2"], inputs["g0"], inputs["be0"],
        inputs["g1"], inputs["be1"], inputs["Wv"], inputs["bv"],
        inputs["Wo"], inputs["bo"], inputs["ln_g"], inputs["ln_b"])

    nc = _get_program(cfg, plan)

    in_maps = []
    for c in range(cfg.C):
        m = dict(wts)
        m.update(per_core[c])
        if plan.icols == 0:
            m["idx"] = np.zeros((P, 16), np.int16)
        in_maps.append(m)

    res = run_bass_kernel_spmd(nc, in_maps, core_ids=list(range(cfg.C)),
                               trace=trace)
    yfull = np.concatenate([res.results[c]["y"].T[:cfg.NLOC]
                            for c in range(cfg.C)], axis=0)
    return yfull.astype(np.float32), res


def kernel(**inputs) -> np.ndarray:
    cfg = Cfg()
    yfull, _ = run(cfg, inputs)
    return yfull

